# revision 22
# baseline (speedup 1.0000x reference)
"""Trainium2 Bass kernel v3 for nn_Att_LSTM_67989332296335.

Live computation (attention softmax is over a singleton axis -> A == 1):
embedding gather -> 1024-step LSTM -> hsum -> Z = tanh(G@Wg1.T + hs@Wg2.T
+ b_g) -> tag = Z@W_tag.T + b_tag -> log_softmax over batch.

v3: the LSTM inner product is computed with W_hh as the STATIONARY
operand (64 FWL-loaded (128,128) bf16 tiles per step, h streams as the
4-column moving operand) in a transposed z.T layout (gate dims on
partitions).  This (a) halves the PE time vs streaming W_hh as the
moving operand, and (b) produces h directly in the hsT layout -- the
per-step h-transpose dance of v2 disappears.

Step schedule ("o7"): per gate block [i, g, f, o], all 4 k-tiles are
contracted consecutively so blocks complete progressively through the
sweep; the elementwise chain (split into batch-independent A/B halves
over h-dims 0:256 / 256:512) walks behind the closing blocks, leaving
only sig(o)+h-mul exposed at sweep end.  X+bias is injected into PSUM
via an identity matmul; bulk-X for the next 128-step chunk is computed
in transposed orientation (W_ih stationary) and drip-fed into the PE
stall windows; its PSUM->SBUF copies ride the scalar engine with the
bias added via the activation bias operand.

Distribution: data-parallel over batch B=32 across 8 cores (B_local=4).
One 8-core AllReduce of the (20, S) exp-sums for the batch log-softmax.
"""
import sys
sys.path.insert(0, "/opt/trn_rl_repo")

import numpy as np
import ml_dtypes

import concourse.bass as bass
import concourse.tile as tile
from concourse import bacc, mybir

F32 = mybir.dt.float32
BF16 = mybir.dt.bfloat16
BF16_NP = ml_dtypes.bfloat16

S, B, E, H, AH, V, T = 1024, 32, 256, 512, 256, 32000, 20
NCORES = 8
BL = B // NCORES          # 4
CH = 128                  # steps per bulk-X chunk

_graph_cache = {}
GPERM = [0, 1, 3, 2]      # column gate-block order becomes [i, f, o, g]
GORDER = (0, 3, 1, 2)     # block processing order: i, g, f, o
SIG = mybir.ActivationFunctionType.Sigmoid
TANH = mybir.ActivationFunctionType.Tanh
IDENT = mybir.ActivationFunctionType.Identity


def _perm_gates_cols(w):
    """Permute the 2048-wide gate axis (last) into [i,f,o,g] block order."""
    shp = w.shape
    v = w.reshape(shp[:-1] + (4, 512))
    return np.ascontiguousarray(v[..., GPERM, :].reshape(shp))


def build_graph(steps=S, debug=False, collective=True):
    nc = bacc.Bacc(None, target_bir_lowering=False, debug=debug)
    sb = steps * BL
    ch = min(CH, steps)
    nchunk = steps // ch
    tpn = min(128, steps)                # timesteps per P3/P4 N-tile
    nw = tpn * BL                        # N width (cols) per tile
    ntb = steps // tpn                   # number of N-tiles
    tw = min(128, sb)                    # output-transpose tile width
    ntr = sb // tw

    def P(name, shape, dt):
        return nc.dram_tensor(name, list(shape), dt, kind="ExternalInput")

    e0_d = P("e0", (128, sb), BF16)
    e1_d = P("e1", (128, sb), BF16)
    whst_d = P("whst", (128, 64 * 128), BF16)
    wih01_d = P("wih01", (128, 2 * 2048), BF16)
    biasT_d = P("biasT", (128, 16), F32)
    i128_d = P("i128", (128, 128), BF16)
    sm32_d = P("sm32", (128, 256), F32)
    bgt_d = P("bgt", (1, 256), F32)
    wg1t_d = P("wg1t", (512, 256), F32)
    wg2t_d = P("wg2t", (512, 256), BF16)
    indrep_d = P("indrep", (4, 512), F32)
    wtagt_d = P("wtagt", (256, 20), BF16)
    i20_d = P("i20", (20, 20), F32)
    btag_d = P("btag", (1, 20), BF16)
    ones512_d = P("ones512", (1, 512), BF16)
    out_d = nc.dram_tensor("out", [sb, T], F32, kind="ExternalOutput")

    with tile.TileContext(nc) as tc:
        with tc.tile_pool(name="persist", bufs=1) as pp, \
             tc.tile_pool(name="dram", bufs=1, space="DRAM") as dp:
            e0 = pp.tile([128, sb], BF16, tag="e0")
            e1 = pp.tile([128, sb], BF16, tag="e1")
            whst = pp.tile([128, 64 * 128], BF16, tag="whst")
            wih01 = pp.tile([128, 2 * 2048], BF16, tag="wih01")
            biasT = pp.tile([128, 16], F32, tag="biasT")
            i128 = pp.tile([128, 128], BF16, tag="i128")
            sm32 = pp.tile([128, 256], F32, tag="sm32")
            bgt = pp.tile([1, 256], F32, tag="bgt")
            wg1t = [pp.tile([128, 256], F32, name=f"wg1t{k}", tag=f"wg1t{k}")
                    for k in range(4)]
            wg2t = [pp.tile([128, 256], BF16, name=f"wg2t{k}", tag=f"wg2t{k}")
                    for k in range(4)]
            indrep = pp.tile([4, 512], F32, tag="indrep")
            wtagt = [pp.tile([128, 20], BF16, name=f"wtagt{k}", tag=f"wtagt{k}")
                     for k in range(2)]
            i20 = pp.tile([20, 20], F32, tag="i20")
            btag = pp.tile([1, 20], BF16, tag="btag")
            ones512 = pp.tile([1, 512], BF16, tag="ones512")

            # state
            hsT = pp.tile([128, steps * 16], BF16, tag="hsT")
            hz = pp.tile([128, 16], BF16, tag="hz")
            XT = [pp.tile([128, ch * 64], BF16, name=f"XT{k}", tag=f"XT{k}")
                  for k in range(min(2, nchunk))]
            # merged A/B chain tiles; A = h-dims 0:256 (cols 0:8 of each pg),
            # B = 8:16. sif = [i_AB 0:16 | o_AB 16:32].
            # c-update via pair-scan: d0 = zc = [0, c] pairs (even cols stay
            # zero forever), d1 = fm = [f_hat, m2] pairs; scan state:
            # s(2j) = f_hat_j, s(2j+1) = f_hat_j*c_j + m2_j = c'_j.
            # out -> W = [f_hat, c'] pairs; odd cols of W copied back to
            # zc's odd cols off the critical path.
            sif = pp.tile([128, 32], F32, tag="sif")
            tg = pp.tile([128, 16], F32, tag="tg")
            zc = pp.tile([128, 32], F32, tag="zc")
            wsc = pp.tile([128, 32], F32, tag="wsc")
            fm = pp.tile([128, 32], F32, tag="fm")
            tcl = pp.tile([128, 16], F32, tag="tcl")

            for t_, src in [(e0, e0_d), (e1, e1_d), (whst, whst_d),
                            (wih01, wih01_d), (biasT, biasT_d),
                            (i128, i128_d), (sm32, sm32_d), (bgt, bgt_d),
                            (indrep, indrep_d), (i20, i20_d),
                            (btag, btag_d), (ones512, ones512_d)]:
                nc.sync.dma_start(t_[:], src[:])
            for k in range(4):
                nc.sync.dma_start(wg1t[k][:], wg1t_d[128 * k:128 * (k + 1), :])
                nc.sync.dma_start(wg2t[k][:], wg2t_d[128 * k:128 * (k + 1), :])
            for k in range(2):
                nc.sync.dma_start(wtagt[k][:], wtagt_d[128 * k:128 * (k + 1), :])
            nc.vector.memset(hz[:], 0.0)
            nc.vector.memset(zc[:], 0.0)

            w4 = whst[:].rearrange("p (mt hb u) -> p mt hb u", mt=16, hb=4)
            wi4 = wih01[:].rearrange("p (e mt u) -> p e mt u", e=2, mt=16)
            xt4 = [x[:].rearrange("p (r reg b) -> p r reg b", reg=16, b=4)
                   for x in XT]

            with tc.tile_pool(name="pg", bufs=1, space="PSUM") as pgp, \
                 tc.tile_pool(name="pb", bufs=2, space="PSUM") as pbp:

                pb_cur = [None]

                PBW = 4 * ch          # bulk psum cols per chunk-tile
                PW = PBW // 4         # cols per copy piece
                RW = ch // 4          # xt4 r-rows per copy piece

                def bulk_mms(cn, mt):
                    pb = pbp.tile([128, PBW], F32, tag="pb",
                                  name=f"pb{cn}_{mt}")
                    nc.tensor.matmul(pb[:], wi4[:, 0, mt],
                                     e0[:, PBW * cn:PBW * (cn + 1)],
                                     start=True, stop=False,
                                     skip_group_check=True)
                    nc.tensor.matmul(pb[:], wi4[:, 1, mt],
                                     e1[:, PBW * cn:PBW * (cn + 1)],
                                     start=False, stop=True,
                                     skip_group_check=True)
                    pb_cur[0] = pb
                    return pb

                def bulk_copy(cn, mt, piece, pb):
                    src = pb[:, PW * piece:PW * (piece + 1)].rearrange(
                        "p (r b) -> p r b", b=4)
                    dst = xt4[cn % 2][:, RW * piece:RW * (piece + 1), mt, :]
                    nc.scalar.activation(dst, src, IDENT,
                                         bias=biasT[:, mt:mt + 1])

                # prologue: bulk-X for chunk 0
                for mt in range(16):
                    pb = bulk_mms(0, mt)
                    for piece in range(4):
                        bulk_copy(0, mt, piece, pb)

                for t in range(steps):
                    c, r = divmod(t, ch)
                    hprev = hz if t == 0 else hsT[:, 16 * (t - 1):16 * t]
                    h4 = hprev.rearrange("p (hb b) -> p hb b", hb=4)
                    cn = c + 1
                    pg = {}
                    for bi, G in enumerate(GORDER):
                        pg[G] = pgp.tile([128, 16], F32, tag=f"pg{G}",
                                         name=f"pg{G}_{t}")
                        nc.tensor.matmul(
                            pg[G][:], i128[:],
                            XT[c % 2][:, 64 * r + 16 * G:64 * r + 16 * G + 16],
                            start=True, stop=False, skip_group_check=True)
                        for hbi in (0, 1):
                            for hbo in range(4):
                                nc.tensor.matmul(
                                    pg[G][:, 4 * hbo:4 * hbo + 4],
                                    w4[:, G * 4 + hbo, hbi], h4[:, hbi],
                                    start=False, stop=False,
                                    skip_group_check=True)
                        if bi == 0 and cn < nchunk and r % 8 == 0:
                            bulk_mms(cn, r // 8)
                        # k23 span hbo-outer: region (G,hbo) closes at its
                        # hbi=3 MM, so A-half regions close before B-half.
                        for hbo in range(4):
                            for hbi in (2, 3):
                                nc.tensor.matmul(
                                    pg[G][:, 4 * hbo:4 * hbo + 4],
                                    w4[:, G * 4 + hbo, hbi], h4[:, hbi],
                                    start=False,
                                    stop=(hbi == 3),
                                    skip_group_check=True)

                    # chain: blocks close in order i(pg0), g(pg3), f(pg1),
                    # o(pg2). A = h-dims 0:256 (psum cols 0:8), B = 8:16.
                    nc.scalar.activation(sif[:, 0:16], pg[0][:, 0:16], SIG)
                    nc.scalar.activation(tg[:], pg[3][:, 0:16], TANH)
                    nc.vector.tensor_mul(
                        fm[:].rearrange("p (j u) -> p j u", u=2)[:, :, 1],
                        sif[:, 0:16], tg[:])
                    nc.scalar.activation(
                        fm[:].rearrange("p (j u) -> p j u", u=2)[:, :, 0],
                        pg[1][:, 0:16], SIG)
                    # c' = f_hat*c + i_hat*tanh(g): pair-scan -> wsc
                    nc.vector.tensor_tensor_scan(
                        wsc[:], zc[:], fm[:], 1.0,
                        mybir.AluOpType.mult, mybir.AluOpType.add)
                    nc.scalar.activation(
                        tcl[:],
                        wsc[:].rearrange("p (j u) -> p j u", u=2)[:, :, 1],
                        TANH)
                    # sig(o) and the h-mul stay A/B-split: o's A regions
                    # close first, so hA lands early for sweep t+1.
                    nc.scalar.activation(sif[:, 16:24], pg[2][:, 0:8], SIG)
                    nc.scalar.activation(sif[:, 24:32], pg[2][:, 8:16], SIG)
                    hw = hsT[:, 16 * t:16 * (t + 1)].rearrange(
                        "p (hb b) -> p hb b", hb=4)
                    for hb0, so, tc_ in ((0, sif[:, 16:24], tcl[:, 0:8]),
                                         (2, sif[:, 24:32], tcl[:, 8:16])):
                        nc.vector.tensor_mul(
                            hw[:, hb0:hb0 + 2],
                            so.rearrange("p (hb b) -> p hb b", hb=2),
                            tc_.rearrange("p (hb b) -> p hb b", hb=2))
                    # restore zc odd cols = c' (off the h critical path)
                    nc.vector.tensor_copy(
                        zc[:].rearrange("p (j u) -> p j u", u=2)[:, :, 1],
                        wsc[:].rearrange("p (j u) -> p j u", u=2)[:, :, 1])
                    # bulk-X copies for the next chunk (scalar idle-fill)
                    if cn < nchunk and r % 2 == 1 and r % 8 < 8:
                        mt, piece = r // 8, (r % 8) // 2
                        bulk_copy(cn, mt, piece, pb_cur[0])

            # ---- Phase 2: hsum and gGb ----
            hsum = pp.tile([128, 16], F32, tag="hsum")
            gGb = pp.tile([4, 256], F32, tag="gGb")
            nc.vector.tensor_reduce(
                hsum[:], hsT[:].rearrange("p (t kb) -> p kb t", kb=16),
                mybir.AxisListType.X, mybir.AluOpType.add)
            with tc.tile_pool(name="p2", bufs=1, space="PSUM") as p2p:
                pg2 = p2p.tile([4, 256], F32, tag="pg2")
                nc.tensor.matmul(pg2[:], sm32[0:1, 0:4], bgt[0:1, 0:256],
                                 start=True, stop=False)
                for k in range(4):
                    nc.tensor.matmul(pg2[:], hsum[:, 4 * k:4 * k + 4],
                                     wg1t[k][:], start=False, stop=(k == 3))
                nc.scalar.copy(gGb[:], pg2[:])

            # ---- Phase 3: Z.T = tanh(Wg2.T.T @ hs.T + gGb bcast) ----
            zt = pp.tile([128, 2 * sb], BF16, tag="zt")
            hs4 = hsT[:].rearrange("p (t k b) -> p k t b", k=4, b=4)
            with tc.tile_pool(name="p3", bufs=2, space="PSUM") as p3p:
                for m in range(2):
                    for n in range(ntb):
                        pz3 = p3p.tile([128, nw], F32, tag="pz3")
                        nc.tensor.matmul(pz3[:], gGb[0:4, 128 * m:128 * (m + 1)],
                                         indrep[0:4, 0:nw], start=True, stop=False)
                        for k in range(4):
                            rhs = hs4[:, k, tpn * n:tpn * (n + 1), :]
                            nc.tensor.matmul(pz3[:],
                                             wg2t[k][:, 128 * m:128 * (m + 1)],
                                             rhs, start=False, stop=(k == 3))
                        nc.scalar.activation(
                            zt[:, m * sb + nw * n: m * sb + nw * (n + 1)],
                            pz3[:], TANH)

            # ---- Phase 4: tag.T = Wtag.T.T @ Z.T + b_tag ----
            tagT = pp.tile([128, sb], F32, tag="tagT")
            with tc.tile_pool(name="p4", bufs=2, space="PSUM") as p4p:
                for n in range(ntb):
                    pz4 = p4p.tile([20, nw], F32, tag="pz4")
                    nc.tensor.matmul(pz4[:], btag[0:1, 0:20],
                                     ones512[0:1, 0:nw], start=True, stop=False)
                    for k in range(2):
                        nc.tensor.matmul(
                            pz4[:], wtagt[k][:],
                            zt[:, k * sb + nw * n: k * sb + nw * (n + 1)],
                            start=False, stop=(k == 1))
                    nc.scalar.copy(tagT[0:20, nw * n:nw * (n + 1)], pz4[:])

            # ---- Phase 5: log-softmax over batch ----
            etag = pp.tile([128, sb], F32, tag="etag")
            sums = pp.tile([128, steps], F32, tag="sums")
            nc.scalar.activation(etag[0:20, :], tagT[0:20, :],
                                 mybir.ActivationFunctionType.Exp)
            nc.vector.tensor_reduce(
                sums[0:20, :],
                etag[0:20, :].rearrange("p (t b) -> p t b", b=BL),
                mybir.AxisListType.X, mybir.AluOpType.add)
            if collective:
                cc_in = dp.tile([20, steps], F32, tag="cc_in")
                cc_out = dp.tile([20, steps], F32, tag="cc_out")
                nc.gpsimd.dma_start(cc_in[:], sums[0:20, :])
                nc.gpsimd.collective_compute(
                    "AllReduce", mybir.AluOpType.add,
                    replica_groups=[list(range(NCORES))],
                    ins=[cc_in[:].opt()], outs=[cc_out[:].opt()])
                nc.gpsimd.dma_start(sums[0:20, :], cc_out[:])
            nc.scalar.activation(sums[0:20, :], sums[0:20, :],
                                 mybir.ActivationFunctionType.Ln)
            tag3 = tagT[0:20, :].rearrange("p (t b) -> p t b", b=BL)
            for b in range(BL):
                nc.vector.tensor_sub(tag3[:, :, b], tag3[:, :, b],
                                     sums[0:20, :])

            # ---- Phase 6: transpose (20, sb) -> (sb, 20) and write out ----
            obuf = pp.tile([tw, ntr * 20], F32, tag="obuf")
            with tc.tile_pool(name="p6", bufs=2, space="PSUM") as p6p:
                for j in range(ntr):
                    pz6 = p6p.tile([tw, 20], F32, tag="pz6")
                    nc.tensor.transpose(pz6[:],
                                        tagT[0:20, tw * j:tw * (j + 1)],
                                        i20[:])
                    nc.scalar.copy(obuf[:, 20 * j:20 * (j + 1)], pz6[:])
            nc.sync.dma_start(
                out_d[:].rearrange("(j p) k -> p j k", p=tw),
                obuf[0:tw, :].rearrange("p (j k) -> p j k", j=ntr))
    nc.finalize()
    return nc


def _prep_inputs(inputs, steps=S):
    """Host-side prep: gather + transpose + pack per-core shards."""
    x = np.asarray(inputs["x"]).astype(np.int64)[:steps]          # (steps, B)
    embed = np.asarray(inputs["embed"], np.float32)
    W_ih = np.asarray(inputs["W_ih"], np.float32)
    W_hh = np.asarray(inputs["W_hh"], np.float32)
    b_ih = np.asarray(inputs["b_ih"], np.float32)
    b_hh = np.asarray(inputs["b_hh"], np.float32)
    W_g = np.asarray(inputs["W_g"], np.float32)
    b_g = np.asarray(inputs["b_g"], np.float32)
    W_tag = np.asarray(inputs["W_tag"], np.float32)
    b_tag = np.asarray(inputs["b_tag"], np.float32)

    sb = steps * BL
    emb = embed[x]                                                # (steps, B, E)
    wihT = _perm_gates_cols(W_ih.T.astype(np.float32))            # (E, 2048)
    bias = _perm_gates_cols((b_ih + b_hh).astype(np.float32))     # (2048,)
    whhT = _perm_gates_cols(W_hh.T.astype(np.float32))            # (512, 2048)

    whst = np.empty((128, 64 * 128), np.float32)
    for mt in range(16):
        for hb in range(4):
            whst[:, (mt * 4 + hb) * 128:(mt * 4 + hb + 1) * 128] = \
                whhT[128 * hb:128 * (hb + 1), 128 * mt:128 * (mt + 1)]
    wih01 = np.empty((128, 2 * 2048), np.float32)
    for e in range(2):
        for mt in range(16):
            wih01[:, e * 2048 + mt * 128:e * 2048 + (mt + 1) * 128] = \
                wihT[128 * e:128 * (e + 1), 128 * mt:128 * (mt + 1)]
    biasT = bias.reshape(16, 128).T.copy()                        # (128, 16)

    sm32 = np.zeros((128, 256), np.float32)
    sm32[0, :4] = 1.0
    shared = {
        "whst": whst.astype(BF16_NP),
        "wih01": wih01.astype(BF16_NP),
        "biasT": biasT,
        "i128": np.eye(128, dtype=BF16_NP),
        "sm32": sm32,
        "bgt": b_g.reshape(1, 256).astype(np.float32),
        "wg1t": W_g[:, :H].T.astype(np.float32).copy(),
        "wg2t": W_g[:, H:].T.astype(BF16_NP),
        "indrep": np.tile(np.eye(4, dtype=np.float32), (1, 128)),
        "wtagt": W_tag.T.astype(BF16_NP),
        "i20": np.eye(20, dtype=np.float32),
        "btag": b_tag.reshape(1, 20).astype(BF16_NP),
        "ones512": np.ones((1, 512), BF16_NP),
    }
    in_maps = []
    for c in range(NCORES):
        sl = emb[:, BL * c:BL * (c + 1), :]                       # (steps, BL, E)
        embT = np.ascontiguousarray(sl.transpose(2, 0, 1).reshape(E, sb))
        m = dict(shared)
        m["e0"] = embT[0:128].astype(BF16_NP)
        m["e1"] = embT[128:256].astype(BF16_NP)
        in_maps.append(m)
    return in_maps


def run(inputs, steps=S, trace=False):
    from concourse.bass_utils import run_bass_kernel_spmd
    key = steps
    if key not in _graph_cache:
        _graph_cache[key] = build_graph(steps)
    nc = _graph_cache[key]
    in_maps = _prep_inputs(inputs, steps)
    res = run_bass_kernel_spmd(nc, in_maps, core_ids=list(range(NCORES)),
                               trace=trace)
    outs = [r["out"].reshape(steps, BL, T) for r in res.results]
    full = np.concatenate(outs, axis=1).astype(np.float32)        # (steps, B, T)
    return full, res


def kernel(**inputs):
    out, _ = run(inputs, steps=S, trace=False)
    return out


# revision 23
# speedup vs baseline: 1.0100x; 1.0100x over previous
"""Trainium2 Bass kernel v3 for nn_Att_LSTM_67989332296335.

Live computation (attention softmax is over a singleton axis -> A == 1):
embedding gather -> 1024-step LSTM -> hsum -> Z = tanh(G@Wg1.T + hs@Wg2.T
+ b_g) -> tag = Z@W_tag.T + b_tag -> log_softmax over batch.

v3: the LSTM inner product is computed with W_hh as the STATIONARY
operand (64 FWL-loaded (128,128) bf16 tiles per step, h streams as the
4-column moving operand) in a transposed z.T layout (gate dims on
partitions).  This (a) halves the PE time vs streaming W_hh as the
moving operand, and (b) produces h directly in the hsT layout -- the
per-step h-transpose dance of v2 disappears.

Step schedule ("o7"): per gate block [i, g, f, o], all 4 k-tiles are
contracted consecutively so blocks complete progressively through the
sweep; the elementwise chain (split into batch-independent A/B halves
over h-dims 0:256 / 256:512) walks behind the closing blocks, leaving
only sig(o)+h-mul exposed at sweep end.  X+bias is injected into PSUM
via an identity matmul; bulk-X for the next 128-step chunk is computed
in transposed orientation (W_ih stationary) and drip-fed into the PE
stall windows; its PSUM->SBUF copies ride the scalar engine with the
bias added via the activation bias operand.

Distribution: data-parallel over batch B=32 across 8 cores (B_local=4).
One 8-core AllReduce of the (20, S) exp-sums for the batch log-softmax.
"""
import sys
sys.path.insert(0, "/opt/trn_rl_repo")

import numpy as np
import ml_dtypes

import concourse.bass as bass
import concourse.tile as tile
from concourse import bacc, mybir

F32 = mybir.dt.float32
BF16 = mybir.dt.bfloat16
BF16_NP = ml_dtypes.bfloat16

S, B, E, H, AH, V, T = 1024, 32, 256, 512, 256, 32000, 20
NCORES = 8
BL = B // NCORES          # 4
CH = 128                  # steps per bulk-X chunk

_graph_cache = {}
GPERM = [0, 1, 3, 2]      # column gate-block order becomes [i, f, o, g]
GORDER = (0, 3, 1, 2)     # block processing order: i, g, f, o
SIG = mybir.ActivationFunctionType.Sigmoid
TANH = mybir.ActivationFunctionType.Tanh
IDENT = mybir.ActivationFunctionType.Identity


def _perm_gates_cols(w):
    """Permute the 2048-wide gate axis (last) into [i,f,o,g] block order."""
    shp = w.shape
    v = w.reshape(shp[:-1] + (4, 512))
    return np.ascontiguousarray(v[..., GPERM, :].reshape(shp))


def build_graph(steps=S, debug=False, collective=True):
    nc = bacc.Bacc(None, target_bir_lowering=False, debug=debug)
    sb = steps * BL
    ch = min(CH, steps)
    nchunk = steps // ch
    tpn = min(128, steps)                # timesteps per P3/P4 N-tile
    nw = tpn * BL                        # N width (cols) per tile
    ntb = steps // tpn                   # number of N-tiles
    tw = min(128, sb)                    # output-transpose tile width
    ntr = sb // tw

    def P(name, shape, dt):
        return nc.dram_tensor(name, list(shape), dt, kind="ExternalInput")

    e0_d = P("e0", (128, sb), BF16)
    e1_d = P("e1", (128, sb), BF16)
    whst_d = P("whst", (128, 64 * 128), BF16)
    wih01_d = P("wih01", (128, 2 * 2048), BF16)
    biasT_d = P("biasT", (128, 16), F32)
    i128_d = P("i128", (128, 128), BF16)
    sm32_d = P("sm32", (128, 256), F32)
    bgt_d = P("bgt", (1, 256), F32)
    wg1t_d = P("wg1t", (512, 256), F32)
    wg2t_d = P("wg2t", (512, 256), BF16)
    indrep_d = P("indrep", (4, 512), F32)
    wtagt_d = P("wtagt", (256, 20), BF16)
    i20_d = P("i20", (20, 20), F32)
    btag_d = P("btag", (1, 20), BF16)
    ones512_d = P("ones512", (1, 512), BF16)
    out_d = nc.dram_tensor("out", [sb, T], F32, kind="ExternalOutput")

    with tile.TileContext(nc) as tc:
        with tc.tile_pool(name="persist", bufs=1) as pp, \
             tc.tile_pool(name="dram", bufs=1, space="DRAM") as dp:
            e0 = pp.tile([128, sb], BF16, tag="e0")
            e1 = pp.tile([128, sb], BF16, tag="e1")
            whst = pp.tile([128, 64 * 128], BF16, tag="whst")
            wih01 = pp.tile([128, 2 * 2048], BF16, tag="wih01")
            biasT = pp.tile([128, 16], F32, tag="biasT")
            i128 = pp.tile([128, 128], BF16, tag="i128")
            sm32 = pp.tile([128, 256], F32, tag="sm32")
            bgt = pp.tile([1, 256], F32, tag="bgt")
            wg1t = [pp.tile([128, 256], F32, name=f"wg1t{k}", tag=f"wg1t{k}")
                    for k in range(4)]
            wg2t = [pp.tile([128, 256], BF16, name=f"wg2t{k}", tag=f"wg2t{k}")
                    for k in range(4)]
            indrep = pp.tile([4, 512], F32, tag="indrep")
            wtagt = [pp.tile([128, 20], BF16, name=f"wtagt{k}", tag=f"wtagt{k}")
                     for k in range(2)]
            i20 = pp.tile([20, 20], F32, tag="i20")
            btag = pp.tile([1, 20], BF16, tag="btag")
            ones512 = pp.tile([1, 512], BF16, tag="ones512")

            # state
            hsT = pp.tile([128, steps * 16], BF16, tag="hsT")
            hz = pp.tile([128, 16], BF16, tag="hz")
            XT = [pp.tile([128, ch * 64], BF16, name=f"XT{k}", tag=f"XT{k}")
                  for k in range(min(2, nchunk))]
            # merged A/B chain tiles; A = h-dims 0:256 (cols 0:8 of each pg),
            # B = 8:16. sif = [i_AB 0:16 | o_AB 16:32].
            # c-update via pair-scan: d0 = zc = [0, c] pairs (even cols stay
            # zero forever), d1 = fm = [f_hat, m2] pairs; scan state:
            # s(2j) = f_hat_j, s(2j+1) = f_hat_j*c_j + m2_j = c'_j.
            # out -> W = [f_hat, c'] pairs; odd cols of W copied back to
            # zc's odd cols off the critical path.
            sif = pp.tile([128, 32], F32, tag="sif")
            tg = pp.tile([128, 16], F32, tag="tg")
            zc = pp.tile([128, 32], F32, tag="zc")
            wsc = pp.tile([128, 32], F32, tag="wsc")
            fm = pp.tile([128, 32], F32, tag="fm")
            tcl = pp.tile([128, 16], F32, tag="tcl")

            for t_, src in [(e0, e0_d), (e1, e1_d), (whst, whst_d),
                            (wih01, wih01_d), (biasT, biasT_d),
                            (i128, i128_d), (sm32, sm32_d), (bgt, bgt_d),
                            (indrep, indrep_d), (i20, i20_d),
                            (btag, btag_d), (ones512, ones512_d)]:
                nc.sync.dma_start(t_[:], src[:])
            for k in range(4):
                nc.sync.dma_start(wg1t[k][:], wg1t_d[128 * k:128 * (k + 1), :])
                nc.sync.dma_start(wg2t[k][:], wg2t_d[128 * k:128 * (k + 1), :])
            for k in range(2):
                nc.sync.dma_start(wtagt[k][:], wtagt_d[128 * k:128 * (k + 1), :])
            nc.vector.memset(hz[:], 0.0)
            nc.vector.memset(zc[:], 0.0)

            w4 = whst[:].rearrange("p (mt hb u) -> p mt hb u", mt=16, hb=4)
            wi4 = wih01[:].rearrange("p (e mt u) -> p e mt u", e=2, mt=16)
            xt4 = [x[:].rearrange("p (r reg b) -> p r reg b", reg=16, b=4)
                   for x in XT]

            with tc.tile_pool(name="pg", bufs=1, space="PSUM") as pgp, \
                 tc.tile_pool(name="pb", bufs=2, space="PSUM") as pbp:

                pb_cur = [None]

                PBW = 4 * ch          # bulk psum cols per chunk-tile
                PW = PBW // 4         # cols per copy piece
                RW = ch // 4          # xt4 r-rows per copy piece

                def bulk_mms(cn, mt):
                    pb = pbp.tile([128, PBW], F32, tag="pb",
                                  name=f"pb{cn}_{mt}")
                    nc.tensor.matmul(pb[:], wi4[:, 0, mt],
                                     e0[:, PBW * cn:PBW * (cn + 1)],
                                     start=True, stop=False,
                                     skip_group_check=True)
                    nc.tensor.matmul(pb[:], wi4[:, 1, mt],
                                     e1[:, PBW * cn:PBW * (cn + 1)],
                                     start=False, stop=True,
                                     skip_group_check=True)
                    pb_cur[0] = pb
                    return pb

                def bulk_copy(cn, mt, piece, pb):
                    src = pb[:, PW * piece:PW * (piece + 1)].rearrange(
                        "p (r b) -> p r b", b=4)
                    dst = xt4[cn % 2][:, RW * piece:RW * (piece + 1), mt, :]
                    nc.scalar.activation(dst, src, IDENT,
                                         bias=biasT[:, mt:mt + 1])

                # prologue: bulk-X for chunk 0
                for mt in range(16):
                    pb = bulk_mms(0, mt)
                    for piece in range(4):
                        bulk_copy(0, mt, piece, pb)

                for t in range(steps):
                    c, r = divmod(t, ch)
                    hprev = hz if t == 0 else hsT[:, 16 * (t - 1):16 * t]
                    h4 = hprev.rearrange("p (hb b) -> p hb b", hb=4)
                    cn = c + 1
                    pg = {}
                    for bi, G in enumerate(GORDER):
                        pg[G] = pgp.tile([128, 16], F32, tag=f"pg{G}",
                                         name=f"pg{G}_{t}")
                        nc.tensor.matmul(
                            pg[G][:], i128[:],
                            XT[c % 2][:, 64 * r + 16 * G:64 * r + 16 * G + 16],
                            start=True, stop=False, skip_group_check=True)
                        for hbi in (0, 1):
                            for hbo in range(4):
                                nc.tensor.matmul(
                                    pg[G][:, 4 * hbo:4 * hbo + 4],
                                    w4[:, G * 4 + hbo, hbi], h4[:, hbi],
                                    start=False, stop=False,
                                    skip_group_check=True)
                        if bi == 0 and cn < nchunk and r % 8 == 0:
                            bulk_mms(cn, r // 8)
                        # k23 span hbo-outer: region (G,hbo) closes at its
                        # hbi=3 MM, so A-half regions close before B-half.
                        for hbo in range(4):
                            for hbi in (2, 3):
                                nc.tensor.matmul(
                                    pg[G][:, 4 * hbo:4 * hbo + 4],
                                    w4[:, G * 4 + hbo, hbi], h4[:, hbi],
                                    start=False,
                                    stop=(hbi == 3),
                                    skip_group_check=True)

                    # chain: blocks close in order i(pg0), g(pg3), f(pg1),
                    # o(pg2). A = h-dims 0:256 (psum cols 0:8), B = 8:16.
                    nc.scalar.activation(sif[:, 0:16], pg[0][:, 0:16], SIG)
                    nc.scalar.activation(tg[:], pg[3][:, 0:16], TANH)
                    nc.vector.tensor_mul(
                        fm[:].rearrange("p (j u) -> p j u", u=2)[:, :, 1],
                        sif[:, 0:16], tg[:])
                    nc.scalar.activation(
                        fm[:].rearrange("p (j u) -> p j u", u=2)[:, :, 0],
                        pg[1][:, 0:16], SIG)
                    # c' = f_hat*c + i_hat*tanh(g): pair-scan -> wsc
                    nc.vector.tensor_tensor_scan(
                        wsc[:], zc[:], fm[:], 1.0,
                        mybir.AluOpType.mult, mybir.AluOpType.add)
                    # tail fully A/B-split and interleaved so hA (which
                    # gates sweep t+1's first MMs) lands as early as
                    # possible: [tanh_cA, sig_oA, mul_hA | B...]
                    hw = hsT[:, 16 * t:16 * (t + 1)].rearrange(
                        "p (hb b) -> p hb b", hb=4)
                    for h0, hb0 in ((0, 0), (16, 2)):
                        nc.scalar.activation(
                            tcl[:, h0 // 2:h0 // 2 + 8],
                            wsc[:, h0:h0 + 16].rearrange(
                                "p (j u) -> p j u", u=2)[:, :, 1],
                            TANH)
                        nc.scalar.activation(
                            sif[:, 16 + h0 // 2:24 + h0 // 2],
                            pg[2][:, h0 // 2:h0 // 2 + 8], SIG)
                        nc.vector.tensor_mul(
                            hw[:, hb0:hb0 + 2],
                            sif[:, 16 + h0 // 2:24 + h0 // 2].rearrange(
                                "p (hb b) -> p hb b", hb=2),
                            tcl[:, h0 // 2:h0 // 2 + 8].rearrange(
                                "p (hb b) -> p hb b", hb=2))
                    # restore zc odd cols = c' (off the h critical path)
                    nc.vector.tensor_copy(
                        zc[:].rearrange("p (j u) -> p j u", u=2)[:, :, 1],
                        wsc[:].rearrange("p (j u) -> p j u", u=2)[:, :, 1])
                    # bulk-X copies for the next chunk (scalar idle-fill)
                    if cn < nchunk and r % 2 == 1 and r % 8 < 8:
                        mt, piece = r // 8, (r % 8) // 2
                        bulk_copy(cn, mt, piece, pb_cur[0])

            # ---- Phase 2: hsum and gGb ----
            hsum = pp.tile([128, 16], F32, tag="hsum")
            gGb = pp.tile([4, 256], F32, tag="gGb")
            nc.vector.tensor_reduce(
                hsum[:], hsT[:].rearrange("p (t kb) -> p kb t", kb=16),
                mybir.AxisListType.X, mybir.AluOpType.add)
            with tc.tile_pool(name="p2", bufs=1, space="PSUM") as p2p:
                pg2 = p2p.tile([4, 256], F32, tag="pg2")
                nc.tensor.matmul(pg2[:], sm32[0:1, 0:4], bgt[0:1, 0:256],
                                 start=True, stop=False)
                for k in range(4):
                    nc.tensor.matmul(pg2[:], hsum[:, 4 * k:4 * k + 4],
                                     wg1t[k][:], start=False, stop=(k == 3))
                nc.scalar.copy(gGb[:], pg2[:])

            # ---- Phase 3: Z.T = tanh(Wg2.T.T @ hs.T + gGb bcast) ----
            zt = pp.tile([128, 2 * sb], BF16, tag="zt")
            hs4 = hsT[:].rearrange("p (t k b) -> p k t b", k=4, b=4)
            with tc.tile_pool(name="p3", bufs=2, space="PSUM") as p3p:
                for m in range(2):
                    for n in range(ntb):
                        pz3 = p3p.tile([128, nw], F32, tag="pz3")
                        nc.tensor.matmul(pz3[:], gGb[0:4, 128 * m:128 * (m + 1)],
                                         indrep[0:4, 0:nw], start=True, stop=False)
                        for k in range(4):
                            rhs = hs4[:, k, tpn * n:tpn * (n + 1), :]
                            nc.tensor.matmul(pz3[:],
                                             wg2t[k][:, 128 * m:128 * (m + 1)],
                                             rhs, start=False, stop=(k == 3))
                        nc.scalar.activation(
                            zt[:, m * sb + nw * n: m * sb + nw * (n + 1)],
                            pz3[:], TANH)

            # ---- Phase 4: tag.T = Wtag.T.T @ Z.T + b_tag ----
            tagT = pp.tile([128, sb], F32, tag="tagT")
            with tc.tile_pool(name="p4", bufs=2, space="PSUM") as p4p:
                for n in range(ntb):
                    pz4 = p4p.tile([20, nw], F32, tag="pz4")
                    nc.tensor.matmul(pz4[:], btag[0:1, 0:20],
                                     ones512[0:1, 0:nw], start=True, stop=False)
                    for k in range(2):
                        nc.tensor.matmul(
                            pz4[:], wtagt[k][:],
                            zt[:, k * sb + nw * n: k * sb + nw * (n + 1)],
                            start=False, stop=(k == 1))
                    nc.scalar.copy(tagT[0:20, nw * n:nw * (n + 1)], pz4[:])

            # ---- Phase 5: log-softmax over batch ----
            etag = pp.tile([128, sb], F32, tag="etag")
            sums = pp.tile([128, steps], F32, tag="sums")
            nc.scalar.activation(etag[0:20, :], tagT[0:20, :],
                                 mybir.ActivationFunctionType.Exp)
            nc.vector.tensor_reduce(
                sums[0:20, :],
                etag[0:20, :].rearrange("p (t b) -> p t b", b=BL),
                mybir.AxisListType.X, mybir.AluOpType.add)
            if collective:
                cc_in = dp.tile([20, steps], F32, tag="cc_in")
                cc_out = dp.tile([20, steps], F32, tag="cc_out")
                nc.gpsimd.dma_start(cc_in[:], sums[0:20, :])
                nc.gpsimd.collective_compute(
                    "AllReduce", mybir.AluOpType.add,
                    replica_groups=[list(range(NCORES))],
                    ins=[cc_in[:].opt()], outs=[cc_out[:].opt()])
                nc.gpsimd.dma_start(sums[0:20, :], cc_out[:])
            nc.scalar.activation(sums[0:20, :], sums[0:20, :],
                                 mybir.ActivationFunctionType.Ln)
            tag3 = tagT[0:20, :].rearrange("p (t b) -> p t b", b=BL)
            for b in range(BL):
                nc.vector.tensor_sub(tag3[:, :, b], tag3[:, :, b],
                                     sums[0:20, :])

            # ---- Phase 6: transpose (20, sb) -> (sb, 20) and write out ----
            obuf = pp.tile([tw, ntr * 20], F32, tag="obuf")
            with tc.tile_pool(name="p6", bufs=2, space="PSUM") as p6p:
                for j in range(ntr):
                    pz6 = p6p.tile([tw, 20], F32, tag="pz6")
                    nc.tensor.transpose(pz6[:],
                                        tagT[0:20, tw * j:tw * (j + 1)],
                                        i20[:])
                    nc.scalar.copy(obuf[:, 20 * j:20 * (j + 1)], pz6[:])
            nc.sync.dma_start(
                out_d[:].rearrange("(j p) k -> p j k", p=tw),
                obuf[0:tw, :].rearrange("p (j k) -> p j k", j=ntr))
    nc.finalize()
    return nc


def _prep_inputs(inputs, steps=S):
    """Host-side prep: gather + transpose + pack per-core shards."""
    x = np.asarray(inputs["x"]).astype(np.int64)[:steps]          # (steps, B)
    embed = np.asarray(inputs["embed"], np.float32)
    W_ih = np.asarray(inputs["W_ih"], np.float32)
    W_hh = np.asarray(inputs["W_hh"], np.float32)
    b_ih = np.asarray(inputs["b_ih"], np.float32)
    b_hh = np.asarray(inputs["b_hh"], np.float32)
    W_g = np.asarray(inputs["W_g"], np.float32)
    b_g = np.asarray(inputs["b_g"], np.float32)
    W_tag = np.asarray(inputs["W_tag"], np.float32)
    b_tag = np.asarray(inputs["b_tag"], np.float32)

    sb = steps * BL
    emb = embed[x]                                                # (steps, B, E)
    wihT = _perm_gates_cols(W_ih.T.astype(np.float32))            # (E, 2048)
    bias = _perm_gates_cols((b_ih + b_hh).astype(np.float32))     # (2048,)
    whhT = _perm_gates_cols(W_hh.T.astype(np.float32))            # (512, 2048)

    whst = np.empty((128, 64 * 128), np.float32)
    for mt in range(16):
        for hb in range(4):
            whst[:, (mt * 4 + hb) * 128:(mt * 4 + hb + 1) * 128] = \
                whhT[128 * hb:128 * (hb + 1), 128 * mt:128 * (mt + 1)]
    wih01 = np.empty((128, 2 * 2048), np.float32)
    for e in range(2):
        for mt in range(16):
            wih01[:, e * 2048 + mt * 128:e * 2048 + (mt + 1) * 128] = \
                wihT[128 * e:128 * (e + 1), 128 * mt:128 * (mt + 1)]
    biasT = bias.reshape(16, 128).T.copy()                        # (128, 16)

    sm32 = np.zeros((128, 256), np.float32)
    sm32[0, :4] = 1.0
    shared = {
        "whst": whst.astype(BF16_NP),
        "wih01": wih01.astype(BF16_NP),
        "biasT": biasT,
        "i128": np.eye(128, dtype=BF16_NP),
        "sm32": sm32,
        "bgt": b_g.reshape(1, 256).astype(np.float32),
        "wg1t": W_g[:, :H].T.astype(np.float32).copy(),
        "wg2t": W_g[:, H:].T.astype(BF16_NP),
        "indrep": np.tile(np.eye(4, dtype=np.float32), (1, 128)),
        "wtagt": W_tag.T.astype(BF16_NP),
        "i20": np.eye(20, dtype=np.float32),
        "btag": b_tag.reshape(1, 20).astype(BF16_NP),
        "ones512": np.ones((1, 512), BF16_NP),
    }
    in_maps = []
    for c in range(NCORES):
        sl = emb[:, BL * c:BL * (c + 1), :]                       # (steps, BL, E)
        embT = np.ascontiguousarray(sl.transpose(2, 0, 1).reshape(E, sb))
        m = dict(shared)
        m["e0"] = embT[0:128].astype(BF16_NP)
        m["e1"] = embT[128:256].astype(BF16_NP)
        in_maps.append(m)
    return in_maps


def run(inputs, steps=S, trace=False):
    from concourse.bass_utils import run_bass_kernel_spmd
    key = steps
    if key not in _graph_cache:
        _graph_cache[key] = build_graph(steps)
    nc = _graph_cache[key]
    in_maps = _prep_inputs(inputs, steps)
    res = run_bass_kernel_spmd(nc, in_maps, core_ids=list(range(NCORES)),
                               trace=trace)
    outs = [r["out"].reshape(steps, BL, T) for r in res.results]
    full = np.concatenate(outs, axis=1).astype(np.float32)        # (steps, B, T)
    return full, res


def kernel(**inputs):
    out, _ = run(inputs, steps=S, trace=False)
    return out
